# revision 44
# baseline (speedup 1.0000x reference)
"""Trainium2 Bass kernel for the MAVE global-epistasis measurement layer.

    y[b] = a_0 + sum_k bk[k] * tanh( (ck @ z[b])[k] + dk[k] )
    z: [2097152, 16] f32, ck: [64, 16], bk, dk: [64], a_0: [1]

Data-parallel over 8 NeuronCores (262144 batch rows per core).

Per-core dataflow (Tile kernel), batch enumerated per super s (4096 rows):
  - z loaded in [128, 512] tiles, partition p = 32 consecutive rows
    (2 KB contiguous per partition -> 128 fat DMA descriptors per tile).
  - DVE 32x32 block transpose -> zt[32a + 16v + z, 32Q + j] =
    z[4096 s + 1024 a + 32 j + 2 Q + v, z]; bitcast to f32r (same bits).
  - TensorE: 4 concurrent row-tiled matmuls (tile_position=(32a,0), K=32)
    against a block-diagonal ck stationary -> pre-h [128=(64v+k), 512].
  - ScalarE: tanh(x + dk) over [128, 1536] PSUM tiles (3 banks) -> bf16.
  - TensorE: bk stationary [128, 32] reduces k; 16 groups (slots)
    accumulate into one y PSUM [32=(2*slot+v), 512] bank.
  - GpSimd adds a_0 (PSUM->SBUF); DVE 32x32-transposes y so each
    partition j holds batch rows {1024 slot + 32 j + (0..31)}; HWDGE
    writes 128-byte contiguous DRAM runs.
"""
import numpy as np

import concourse.bass as bass
import concourse.tile as tile
from concourse import mybir
from concourse.bass_utils import run_bass_kernel_spmd

from contextlib import ExitStack

F32 = mybir.dt.float32
F32R = mybir.dt.float32r
BF16 = mybir.dt.bfloat16
U32 = mybir.dt.uint32

B_FULL = 2097152
N_CORES = 8
NC_ROWS = B_FULL // N_CORES          # 262144
SUPER = 4096                         # rows per transpose tile
N_SUPER = NC_ROWS // SUPER           # 64
N_GROUPS = N_SUPER * 4               # 256  (1024 rows each)
SPAN = 16384                         # rows per y flush (16 groups)
N_SPAN = NC_ROWS // SPAN             # 16
HTILE = 4                            # groups per ACT tanh op (4 PSUM banks)


def _multiwait_split(nc):
    ctr = 0
    for f in nc.m.functions:
        for blk in f.blocks:
            insts = blk.instructions
            i = 0
            while i < len(insts):
                inst = insts[i]
                si = getattr(inst, "sync_info", None)
                if si is not None and si.on_wait and len(si.on_wait) > 1:
                    extra = list(si.on_wait[:-1])
                    del si.on_wait[:-1]
                    for w in extra:
                        ctr += 1
                        nop = mybir.InstNoOp(name=f"I-mws-{ctr}", ins=[], outs=[])
                        nop.engine = inst.engine
                        nop.sync_info = mybir.SyncInfo(on_wait=[w], on_update=[])
                        insts.insert(i, nop)
                        i += 1
                i += 1
    return nc


def build_nc():
    nc = bass.Bass()
    z_ext = nc.declare_dram_parameter("z", [NC_ROWS, 16], F32, isOutput=False)
    # Host-precomputed constant tiles (see _run): contiguous, cast-free loads.
    ckbd_ext = nc.declare_dram_parameter("ckbd", [128, 128], BF16, isOutput=False)
    bwide_ext = nc.declare_dram_parameter("bwide", [128, 64], BF16, isOutput=False)
    dkc_ext = nc.declare_dram_parameter("dk_col", [128, 1], F32, isOutput=False)
    a0c_ext = nc.declare_dram_parameter("a0_col", [32, 1], F32, isOutput=False)
    y_ext = nc.declare_dram_parameter("y", [NC_ROWS, 1], F32, isOutput=True)

    ctx = ExitStack()
    with ctx:
        tc = ctx.enter_context(tile.TileContext(nc))
        consts = ctx.enter_context(tc.tile_pool(name="consts", bufs=1))
        zn_pool = ctx.enter_context(tc.tile_pool(name="zn", bufs=4))
        ztr_pool = ctx.enter_context(tc.tile_pool(name="ztr", bufs=4))
        hsb_pool = ctx.enter_context(tc.tile_pool(name="hsb", bufs=3))
        hps4_pool = ctx.enter_context(tc.tile_pool(name="hps4", bufs=1, space="PSUM"))
        hps3_pool = ctx.enter_context(tc.tile_pool(name="hps3", bufs=1, space="PSUM"))
        yps_pool = ctx.enter_context(tc.tile_pool(name="yps", bufs=1, space="PSUM"))

        # DRAM views ------------------------------------------------------
        # z input: partition p of super s holds rows [4096 s + 32 p, +32),
        # i.e. 2 KB contiguous per partition.
        zd = z_ext[:].rearrange("(s p r) z -> s p (r z)", s=N_SUPER, p=128, r=32)
        # row = 16384 t + 1024 slot + 32 j + w, where w = 2Q + v
        yd = y_ext[:].rearrange(
            "(t slot j w) one -> t j slot (w one)",
            t=N_SPAN, slot=16, j=32, w=32,
        )

        zb_tiles = {}

        def load_super(s):
            zb = zn_pool.tile([128, 512], BF16, tag="zb")
            nc.gpsimd.dma_start(out=zb, in_=zd[s])
            zb_tiles[s] = zb

        # ---- constants: host-precomputed, 4 contiguous cast-free loads.
        ckbd = consts.tile([128, 128], BF16, tag="ckbd")
        nc.sync.dma_start(out=ckbd, in_=ckbd_ext[:])
        bwide = consts.tile([128, 64], BF16, tag="bwide")
        nc.sync.dma_start(out=bwide, in_=bwide_ext[:])
        # slot s uses bwide cols [30-2s, 62-2s): bk sits at cols 30 (v=0
        # rows 0-63) / 31 (v=1 rows 64-127) -> slice cols 2s/2s+1 -> y
        # partitions 2s/2s+1.
        bw = [bwide[:, 30 - 2 * s: 62 - 2 * s] for s in range(16)]
        dk_col = consts.tile([128, 1], F32, tag="dkcol")
        nc.sync.dma_start(out=dk_col, in_=dkc_ext[:])
        a0_col = consts.tile([32, 1], F32, tag="a0col")
        nc.sync.dma_start(out=a0_col, in_=a0c_ext[:])

        # Force the tanh ACT table load during startup (it is otherwise
        # lazily loaded at the first real activation, ~1.3 us mid-pipeline).
        tanh_warm = consts.tile([128, 1], BF16, tag="tanhwarm")
        nc.scalar.activation(tanh_warm, dk_col,
                             mybir.ActivationFunctionType.Tanh,
                             bias=dk_col, scale=1.0)

        # ---- z prefetch
        for s in range(4):
            load_super(s)

        # ---- y accumulators: spans alternate between the two partition
        # halves of single [64, 512] buffers (one PSUM bank total).
        ypair = yps_pool.tile([64, 512], F32, tag="ypair")
        ytr_pair = consts.tile([64, 512], F32, tag="ytrpair")
        yfin_pair = consts.tile([64, 512], F32, tag="yfinpair")

        # ---- main loop ---------------------------------------------------
        h_ps = h_sb = y_ps = None
        pend = []
        cur, use4 = 0, True

        def flush_act():
            nonlocal pend, y_ps
            if not pend:
                return
            ncols = len(pend) * 512
            nc.scalar.activation(
                h_sb[:, :ncols], h_ps[:, :ncols],
                mybir.ActivationFunctionType.Tanh,
                bias=dk_col, scale=1.0,
            )
            for gg, col in pend:
                slot = gg % 16
                t = gg // 16
                hh = 32 * (t % 2)
                if slot == 0:
                    y_ps = ypair[hh:hh + 32, :]
                nc.tensor.matmul(
                    y_ps, bw[slot], h_sb[:, col:col + 512],
                    start=(slot == 0), stop=(slot == 15),
                )
                if slot == 15:
                    # y_tr[j, 32 Q + 2 slot + v] = y_ps[2 slot + v, 32 Q + j]
                    y_tr = ytr_pair[hh:hh + 32, :]
                    nc.vector.transpose(y_tr, y_ps)
                    # y_fin[j, 32 slot + 2 Q + v] = y_tr[j, 32 Q + 2 slot + v] + a0
                    y_fin = yfin_pair[hh:hh + 32, :]
                    nc.vector.tensor_scalar_add(
                        y_fin.rearrange("j (slot Q v) -> j slot Q v",
                                        slot=16, Q=16, v=2),
                        y_tr.rearrange("j (Q slot v) -> j slot Q v",
                                       Q=16, slot=16, v=2),
                        a0_col,
                    )
                    nc.sync.dma_start(
                        out=yd[t],
                        in_=y_fin.rearrange("j (slot w) -> j slot w",
                                            slot=16, w=32),
                    )
            pend = []

        for g in range(N_GROUPS):
            s, a = divmod(g, 4)
            if a == 0:
                if s not in zb_tiles:
                    load_super(s)
                zb = zb_tiles.pop(s)
                if s + 4 < N_SUPER:
                    load_super(s + 4)
                zt = ztr_pool.tile([128, 512], BF16)
                nc.vector.transpose(zt, zb)
            if cur == 0:
                cap = 4 if use4 else 3
                pool = hps4_pool if use4 else hps3_pool
                h_ps = pool.tile([128, cap * 512], F32)
                h_sb = hsb_pool.tile([128, 2048], BF16)
            col = cur * 512
            nc.tensor.matmul(
                h_ps[:, col:col + 512],
                ckbd[32 * a:32 * a + 32, :],
                zt[32 * a:32 * a + 32, :],
                start=True, stop=True,
                tile_position=(32 * a, 0),
            )
            pend.append((g, col))
            cur += 1
            if cur == cap or g == N_GROUPS - 1:
                flush_act()
                cur, use4 = 0, not use4

    _multiwait_split(nc)
    return nc


_NC_CACHE = None


def _get_nc():
    global _NC_CACHE
    if _NC_CACHE is None:
        _NC_CACHE = build_nc()
    return _NC_CACHE


def _run(inputs, **run_kwargs):
    import ml_dtypes

    nc = _get_nc()
    bf16 = ml_dtypes.bfloat16
    z = np.ascontiguousarray(np.asarray(inputs["z"], dtype=np.float32))
    a0 = np.asarray(inputs["a_0"], dtype=np.float32).reshape(1)
    bk = np.asarray(inputs["bk"], dtype=np.float32).reshape(64)
    ck = np.ascontiguousarray(np.asarray(inputs["ck"], dtype=np.float32))
    dk = np.asarray(inputs["dk"], dtype=np.float32).reshape(64)

    # Host-side constant prep (tiny): block-diagonal ck stationary, wide bk
    # stationary, replicated dk bias column, broadcast a0.
    ckbd = np.zeros((128, 128), dtype=bf16)
    for a in range(4):
        for v in range(2):
            ckbd[32 * a + 16 * v: 32 * a + 16 * v + 16,
                 64 * v: 64 * v + 64] = ck.T.astype(bf16)
    bwide = np.zeros((128, 64), dtype=bf16)
    bwide[0:64, 30] = bk.astype(bf16)
    bwide[64:128, 31] = bk.astype(bf16)
    dk_col = np.concatenate([dk, dk]).reshape(128, 1)
    a0_col = np.broadcast_to(a0.reshape(1, 1), (32, 1)).copy()
    in_maps = []
    for c in range(N_CORES):
        in_maps.append({
            "z": z[c * NC_ROWS:(c + 1) * NC_ROWS],
            "ckbd": ckbd, "bwide": bwide,
            "dk_col": dk_col, "a0_col": a0_col,
        })
    res = run_bass_kernel_spmd(nc, in_maps, core_ids=list(range(N_CORES)),
                               **run_kwargs)
    y = np.concatenate([res.results[c]["y"] for c in range(N_CORES)], axis=0)
    return y, res


def kernel(**inputs) -> np.ndarray:
    y, _ = _run(inputs)
    return y


# revision 45
# speedup vs baseline: 1.4097x; 1.4097x over previous
"""Trainium2 Bass kernel for the MAVE global-epistasis measurement layer.

    y[b] = a_0 + sum_k bk[k] * tanh( (ck @ z[b])[k] + dk[k] )
    z: [2097152, 16] f32, ck: [64, 16], bk, dk: [64], a_0: [1]

Data-parallel over 8 NeuronCores (262144 batch rows per core).

Per-core dataflow (Tile kernel), batch enumerated per super s (4096 rows):
  - z loaded in [128, 512] tiles, partition p = 32 consecutive rows
    (2 KB contiguous per partition -> 128 fat DMA descriptors per tile).
  - DVE 32x32 block transpose -> zt[32a + 16v + z, 32Q + j] =
    z[4096 s + 1024 a + 32 j + 2 Q + v, z]; bitcast to f32r (same bits).
  - TensorE: 4 concurrent row-tiled matmuls (tile_position=(32a,0), K=32)
    against a block-diagonal ck stationary -> pre-h [128=(64v+k), 512].
  - ScalarE: tanh(x + dk) over [128, 1536] PSUM tiles (3 banks) -> bf16.
  - TensorE: bk stationary [128, 32] reduces k; 16 groups (slots)
    accumulate into one y PSUM [32=(2*slot+v), 512] bank.
  - GpSimd adds a_0 (PSUM->SBUF); DVE 32x32-transposes y so each
    partition j holds batch rows {1024 slot + 32 j + (0..31)}; HWDGE
    writes 128-byte contiguous DRAM runs.
"""
import numpy as np

import concourse.bass as bass
import concourse.tile as tile
from concourse import mybir
from concourse.bass_utils import run_bass_kernel_spmd

from contextlib import ExitStack

F32 = mybir.dt.float32
F32R = mybir.dt.float32r
BF16 = mybir.dt.bfloat16
U32 = mybir.dt.uint32

B_FULL = 2097152
N_CORES = 8
NC_ROWS = B_FULL // N_CORES          # 262144
SUPER = 4096                         # rows per transpose tile
N_SUPER = NC_ROWS // SUPER           # 64
N_GROUPS = N_SUPER * 4               # 256  (1024 rows each)
SPAN = 16384                         # rows per y flush (16 groups)
N_SPAN = NC_ROWS // SPAN             # 16
HTILE = 3                            # groups per ACT tanh op (3 PSUM banks)


def _multiwait_split(nc):
    ctr = 0
    for f in nc.m.functions:
        for blk in f.blocks:
            insts = blk.instructions
            i = 0
            while i < len(insts):
                inst = insts[i]
                si = getattr(inst, "sync_info", None)
                if si is not None and si.on_wait and len(si.on_wait) > 1:
                    extra = list(si.on_wait[:-1])
                    del si.on_wait[:-1]
                    for w in extra:
                        ctr += 1
                        nop = mybir.InstNoOp(name=f"I-mws-{ctr}", ins=[], outs=[])
                        nop.engine = inst.engine
                        nop.sync_info = mybir.SyncInfo(on_wait=[w], on_update=[])
                        insts.insert(i, nop)
                        i += 1
                i += 1
    return nc


def build_nc():
    nc = bass.Bass()
    z_ext = nc.declare_dram_parameter("z", [NC_ROWS, 16], F32, isOutput=False)
    # Host-precomputed constant tiles (see _run): contiguous, cast-free loads.
    ckbd_ext = nc.declare_dram_parameter("ckbd", [128, 128], BF16, isOutput=False)
    bwide_ext = nc.declare_dram_parameter("bwide", [128, 64], BF16, isOutput=False)
    dkc_ext = nc.declare_dram_parameter("dk_col", [128, 1], F32, isOutput=False)
    a0c_ext = nc.declare_dram_parameter("a0_col", [32, 1], F32, isOutput=False)
    y_ext = nc.declare_dram_parameter("y", [NC_ROWS, 1], F32, isOutput=True)

    ctx = ExitStack()
    with ctx:
        tc = ctx.enter_context(tile.TileContext(nc))
        consts = ctx.enter_context(tc.tile_pool(name="consts", bufs=1))
        zn_pool = ctx.enter_context(tc.tile_pool(name="zn", bufs=4))
        ztr_pool = ctx.enter_context(tc.tile_pool(name="ztr", bufs=4))
        hsb_pool = ctx.enter_context(tc.tile_pool(name="hsb", bufs=3))
        hps_pool = ctx.enter_context(tc.tile_pool(name="hps", bufs=2, space="PSUM"))
        yps_pool = ctx.enter_context(tc.tile_pool(name="yps", bufs=1, space="PSUM"))

        # DRAM views ------------------------------------------------------
        # z input: partition p of super s holds rows [4096 s + 32 p, +32),
        # i.e. 2 KB contiguous per partition.
        zd = z_ext[:].rearrange("(s p r) z -> s p (r z)", s=N_SUPER, p=128, r=32)
        # row = 16384 t + 1024 slot + 32 j + w, where w = 2Q + v
        yd = y_ext[:].rearrange(
            "(t slot j w) one -> t j slot (w one)",
            t=N_SPAN, slot=16, j=32, w=32,
        )

        zb_tiles = {}

        def load_super(s):
            zb = zn_pool.tile([128, 512], BF16, tag="zb")
            nc.gpsimd.dma_start(out=zb, in_=zd[s])
            zb_tiles[s] = zb

        # ---- constants: host-precomputed, 4 contiguous cast-free loads.
        ckbd = consts.tile([128, 128], BF16, tag="ckbd")
        nc.sync.dma_start(out=ckbd, in_=ckbd_ext[:])
        bwide = consts.tile([128, 64], BF16, tag="bwide")
        nc.sync.dma_start(out=bwide, in_=bwide_ext[:])
        # slot s uses bwide cols [30-2s, 62-2s): bk sits at cols 30 (v=0
        # rows 0-63) / 31 (v=1 rows 64-127) -> slice cols 2s/2s+1 -> y
        # partitions 2s/2s+1.
        bw = [bwide[:, 30 - 2 * s: 62 - 2 * s] for s in range(16)]
        dk_col = consts.tile([128, 1], F32, tag="dkcol")
        nc.sync.dma_start(out=dk_col, in_=dkc_ext[:])
        a0_col = consts.tile([32, 1], F32, tag="a0col")
        nc.sync.dma_start(out=a0_col, in_=a0c_ext[:])

        # Force the tanh ACT table load during startup (it is otherwise
        # lazily loaded at the first real activation, ~1.3 us mid-pipeline).
        tanh_warm = consts.tile([128, 1], BF16, tag="tanhwarm")
        nc.scalar.activation(tanh_warm, dk_col,
                             mybir.ActivationFunctionType.Tanh,
                             bias=dk_col, scale=1.0)

        # ---- z prefetch
        for s in range(4):
            load_super(s)

        # ---- y accumulators: spans alternate between the two partition
        # halves of single [64, 512] buffers (one PSUM bank total).
        ypair = yps_pool.tile([64, 512], F32, tag="ypair")
        ytr_pair = consts.tile([64, 512], F32, tag="ytrpair")
        yfin_pair = consts.tile([64, 512], F32, tag="yfinpair")

        # ---- main loop ---------------------------------------------------
        h_ps = h_sb = y_ps = None
        pend = []

        def flush_act():
            nonlocal pend, y_ps
            if not pend:
                return
            ncols = len(pend) * 512
            nc.scalar.activation(
                h_sb[:, :ncols], h_ps[:, :ncols],
                mybir.ActivationFunctionType.Tanh,
                bias=dk_col, scale=1.0,
            )
            for gg, col in pend:
                slot = gg % 16
                t = gg // 16
                hh = 32 * (t % 2)
                if slot == 0:
                    y_ps = ypair[hh:hh + 32, :]
                nc.tensor.matmul(
                    y_ps, bw[slot], h_sb[:, col:col + 512],
                    start=(slot == 0), stop=(slot == 15),
                )
                if slot == 15:
                    # y_tr[j, 32 Q + 2 slot + v] = y_ps[2 slot + v, 32 Q + j]
                    y_tr = ytr_pair[hh:hh + 32, :]
                    nc.vector.transpose(y_tr, y_ps)
                    # y_fin[j, 32 slot + 2 Q + v] = y_tr[j, 32 Q + 2 slot + v] + a0
                    y_fin = yfin_pair[hh:hh + 32, :]
                    nc.vector.tensor_scalar_add(
                        y_fin.rearrange("j (slot Q v) -> j slot Q v",
                                        slot=16, Q=16, v=2),
                        y_tr.rearrange("j (Q slot v) -> j slot Q v",
                                       Q=16, slot=16, v=2),
                        a0_col,
                    )
                    nc.sync.dma_start(
                        out=yd[t],
                        in_=y_fin.rearrange("j (slot w) -> j slot w",
                                            slot=16, w=32),
                    )
            pend = []

        for g in range(N_GROUPS):
            s, a = divmod(g, 4)
            if a == 0:
                if s not in zb_tiles:
                    load_super(s)
                zb = zb_tiles.pop(s)
                if s + 4 < N_SUPER:
                    load_super(s + 4)
                zt = ztr_pool.tile([128, 512], BF16)
                nc.vector.transpose(zt, zb)
            if g % HTILE == 0:
                h_ps = hps_pool.tile([128, HTILE * 512], F32)
                h_sb = hsb_pool.tile([128, HTILE * 512], BF16)
            col = (g % HTILE) * 512
            nc.tensor.matmul(
                h_ps[:, col:col + 512],
                ckbd[32 * a:32 * a + 32, :],
                zt[32 * a:32 * a + 32, :],
                start=True, stop=True,
                tile_position=(32 * a, 0),
            )
            pend.append((g, col))
            if g % HTILE == HTILE - 1 or g == N_GROUPS - 1:
                flush_act()

    _multiwait_split(nc)
    return nc


_NC_CACHE = None


def _get_nc():
    global _NC_CACHE
    if _NC_CACHE is None:
        _NC_CACHE = build_nc()
    return _NC_CACHE


def _run(inputs, **run_kwargs):
    import ml_dtypes

    nc = _get_nc()
    bf16 = ml_dtypes.bfloat16
    z = np.ascontiguousarray(np.asarray(inputs["z"], dtype=np.float32))
    a0 = np.asarray(inputs["a_0"], dtype=np.float32).reshape(1)
    bk = np.asarray(inputs["bk"], dtype=np.float32).reshape(64)
    ck = np.ascontiguousarray(np.asarray(inputs["ck"], dtype=np.float32))
    dk = np.asarray(inputs["dk"], dtype=np.float32).reshape(64)

    # Host-side constant prep (tiny): block-diagonal ck stationary, wide bk
    # stationary, replicated dk bias column, broadcast a0.
    ckbd = np.zeros((128, 128), dtype=bf16)
    for a in range(4):
        for v in range(2):
            ckbd[32 * a + 16 * v: 32 * a + 16 * v + 16,
                 64 * v: 64 * v + 64] = ck.T.astype(bf16)
    bwide = np.zeros((128, 64), dtype=bf16)
    bwide[0:64, 30] = bk.astype(bf16)
    bwide[64:128, 31] = bk.astype(bf16)
    dk_col = np.concatenate([dk, dk]).reshape(128, 1)
    a0_col = np.broadcast_to(a0.reshape(1, 1), (32, 1)).copy()
    in_maps = []
    for c in range(N_CORES):
        in_maps.append({
            "z": z[c * NC_ROWS:(c + 1) * NC_ROWS],
            "ckbd": ckbd, "bwide": bwide,
            "dk_col": dk_col, "a0_col": a0_col,
        })
    res = run_bass_kernel_spmd(nc, in_maps, core_ids=list(range(N_CORES)),
                               **run_kwargs)
    y = np.concatenate([res.results[c]["y"] for c in range(N_CORES)], axis=0)
    return y, res


def kernel(**inputs) -> np.ndarray:
    y, _ = _run(inputs)
    return y


# revision 46
# speedup vs baseline: 1.4481x; 1.0273x over previous
"""Trainium2 Bass kernel for the MAVE global-epistasis measurement layer.

    y[b] = a_0 + sum_k bk[k] * tanh( (ck @ z[b])[k] + dk[k] )
    z: [2097152, 16] f32, ck: [64, 16], bk, dk: [64], a_0: [1]

Data-parallel over 8 NeuronCores (262144 batch rows per core).

Per-core dataflow (Tile kernel), batch enumerated per super s (4096 rows):
  - z loaded in [128, 512] tiles, partition p = 32 consecutive rows
    (2 KB contiguous per partition -> 128 fat DMA descriptors per tile).
  - DVE 32x32 block transpose -> zt[32a + 16v + z, 32Q + j] =
    z[4096 s + 1024 a + 32 j + 2 Q + v, z]; bitcast to f32r (same bits).
  - TensorE: 4 concurrent row-tiled matmuls (tile_position=(32a,0), K=32)
    against a block-diagonal ck stationary -> pre-h [128=(64v+k), 512].
  - ScalarE: tanh(x + dk) over [128, 1536] PSUM tiles (3 banks) -> bf16.
  - TensorE: bk stationary [128, 32] reduces k; 16 groups (slots)
    accumulate into one y PSUM [32=(2*slot+v), 512] bank.
  - GpSimd adds a_0 (PSUM->SBUF); DVE 32x32-transposes y so each
    partition j holds batch rows {1024 slot + 32 j + (0..31)}; HWDGE
    writes 128-byte contiguous DRAM runs.
"""
import numpy as np

import concourse.bass as bass
import concourse.tile as tile
from concourse import mybir
from concourse.bass_utils import run_bass_kernel_spmd

from contextlib import ExitStack

F32 = mybir.dt.float32
F32R = mybir.dt.float32r
BF16 = mybir.dt.bfloat16
U32 = mybir.dt.uint32

B_FULL = 2097152
N_CORES = 8
NC_ROWS = B_FULL // N_CORES          # 262144
SUPER = 4096                         # rows per transpose tile
N_SUPER = NC_ROWS // SUPER           # 64
N_GROUPS = N_SUPER * 4               # 256  (1024 rows each)
SPAN = 16384                         # rows per y flush (16 groups)
N_SPAN = NC_ROWS // SPAN             # 16
HTILE = 3                            # groups per ACT tanh op (3 PSUM banks)


def _multiwait_split(nc):
    ctr = 0
    for f in nc.m.functions:
        for blk in f.blocks:
            insts = blk.instructions
            i = 0
            while i < len(insts):
                inst = insts[i]
                si = getattr(inst, "sync_info", None)
                if si is not None and si.on_wait and len(si.on_wait) > 1:
                    extra = list(si.on_wait[:-1])
                    del si.on_wait[:-1]
                    for w in extra:
                        ctr += 1
                        nop = mybir.InstNoOp(name=f"I-mws-{ctr}", ins=[], outs=[])
                        nop.engine = inst.engine
                        nop.sync_info = mybir.SyncInfo(on_wait=[w], on_update=[])
                        insts.insert(i, nop)
                        i += 1
                i += 1
    return nc


def build_nc():
    nc = bass.Bass()
    z_ext = nc.declare_dram_parameter("z", [NC_ROWS, 16], F32, isOutput=False)
    # Host-precomputed constant tiles (see _run): contiguous, cast-free loads.
    ckbd_ext = nc.declare_dram_parameter("ckbd", [128, 128], BF16, isOutput=False)
    bwide_ext = nc.declare_dram_parameter("bwide", [128, 64], BF16, isOutput=False)
    dkc_ext = nc.declare_dram_parameter("dk_col", [128, 1], F32, isOutput=False)
    a0c_ext = nc.declare_dram_parameter("a0_col", [32, 1], F32, isOutput=False)
    y_ext = nc.declare_dram_parameter("y", [NC_ROWS, 1], F32, isOutput=True)

    ctx = ExitStack()
    with ctx:
        tc = ctx.enter_context(tile.TileContext(nc))
        consts = ctx.enter_context(tc.tile_pool(name="consts", bufs=1))
        zn_pool = ctx.enter_context(tc.tile_pool(name="zn", bufs=4))
        ztr_pool = ctx.enter_context(tc.tile_pool(name="ztr", bufs=4))
        hsb_pool = ctx.enter_context(tc.tile_pool(name="hsb", bufs=3))
        ysb_pool = ctx.enter_context(tc.tile_pool(name="ysb", bufs=2))
        yt_pool = ctx.enter_context(tc.tile_pool(name="yt", bufs=2))
        hps_pool = ctx.enter_context(tc.tile_pool(name="hps", bufs=2, space="PSUM"))
        yps_pool = ctx.enter_context(tc.tile_pool(name="yps", bufs=2, space="PSUM"))

        # DRAM views ------------------------------------------------------
        # z input: partition p of super s holds rows [4096 s + 32 p, +32),
        # i.e. 2 KB contiguous per partition.
        zd = z_ext[:].rearrange("(s p r) z -> s p (r z)", s=N_SUPER, p=128, r=32)
        # row = 16384 t + 1024 slot + 32 j + w, where w = 2Q + v
        yd = y_ext[:].rearrange(
            "(t slot j w) one -> t j slot (w one)",
            t=N_SPAN, slot=16, j=32, w=32,
        )

        zb_tiles = {}

        def load_super(s):
            zb = zn_pool.tile([128, 512], BF16, tag="zb")
            nc.gpsimd.dma_start(out=zb, in_=zd[s])
            zb_tiles[s] = zb

        # ---- constants: host-precomputed, 4 contiguous cast-free loads.
        ckbd = consts.tile([128, 128], BF16, tag="ckbd")
        nc.sync.dma_start(out=ckbd, in_=ckbd_ext[:])
        bwide = consts.tile([128, 64], BF16, tag="bwide")
        nc.sync.dma_start(out=bwide, in_=bwide_ext[:])
        # slot s uses bwide cols [30-2s, 62-2s): bk sits at cols 30 (v=0
        # rows 0-63) / 31 (v=1 rows 64-127) -> slice cols 2s/2s+1 -> y
        # partitions 2s/2s+1.
        bw = [bwide[:, 30 - 2 * s: 62 - 2 * s] for s in range(16)]
        dk_col = consts.tile([128, 1], F32, tag="dkcol")
        nc.sync.dma_start(out=dk_col, in_=dkc_ext[:])
        a0_col = consts.tile([32, 1], F32, tag="a0col")
        nc.sync.dma_start(out=a0_col, in_=a0c_ext[:])

        # Force the tanh ACT table load during startup (it is otherwise
        # lazily loaded at the first real activation, ~1.3 us mid-pipeline).
        tanh_warm = consts.tile([128, 1], BF16, tag="tanhwarm")
        nc.scalar.activation(tanh_warm, dk_col,
                             mybir.ActivationFunctionType.Tanh,
                             bias=dk_col, scale=1.0)

        # ---- z prefetch
        for s in range(4):
            load_super(s)

        # ---- main loop ---------------------------------------------------
        h_ps = h_sb = y_ps = None
        pend = []

        def flush_act():
            nonlocal pend, y_ps
            if not pend:
                return
            ncols = len(pend) * 512
            nc.scalar.activation(
                h_sb[:, :ncols], h_ps[:, :ncols],
                mybir.ActivationFunctionType.Tanh,
                bias=dk_col, scale=1.0,
            )
            for gg, col in pend:
                slot = gg % 16
                t = gg // 16
                if slot == 0:
                    y_ps = yps_pool.tile([32, 512], F32, tag="y_ps")
                nc.tensor.matmul(
                    y_ps, bw[slot], h_sb[:, col:col + 512],
                    start=(slot == 0), stop=(slot == 15),
                )
                if slot == 15:
                    # y_tr[j, 32 Q + 2 slot + v] = y_ps[2 slot + v, 32 Q + j]
                    y_tr = yt_pool.tile([32, 512], F32, tag="y_tr")
                    nc.vector.transpose(y_tr, y_ps)
                    # y_fin[j, 32 slot + 2 Q + v] = y_tr[j, 32 Q + 2 slot + v] + a0
                    y_fin = ysb_pool.tile([32, 512], F32, tag="y_fin")
                    nc.vector.tensor_scalar_add(
                        y_fin.rearrange("j (slot Q v) -> j slot Q v",
                                        slot=16, Q=16, v=2),
                        y_tr.rearrange("j (Q slot v) -> j slot Q v",
                                       Q=16, slot=16, v=2),
                        a0_col,
                    )
                    nc.sync.dma_start(
                        out=yd[t],
                        in_=y_fin.rearrange("j (slot w) -> j slot w",
                                            slot=16, w=32),
                    )
            pend = []

        for g in range(N_GROUPS):
            s, a = divmod(g, 4)
            if a == 0:
                if s not in zb_tiles:
                    load_super(s)
                zb = zb_tiles.pop(s)
                if s + 4 < N_SUPER:
                    load_super(s + 4)
                zt = ztr_pool.tile([128, 512], BF16)
                nc.vector.transpose(zt, zb)
            if g % HTILE == 0:
                h_ps = hps_pool.tile([128, HTILE * 512], F32)
                h_sb = hsb_pool.tile([128, HTILE * 512], BF16)
            col = (g % HTILE) * 512
            nc.tensor.matmul(
                h_ps[:, col:col + 512],
                ckbd[32 * a:32 * a + 32, :],
                zt[32 * a:32 * a + 32, :],
                start=True, stop=True,
                tile_position=(32 * a, 0),
            )
            pend.append((g, col))
            if g % HTILE == HTILE - 1 or g == N_GROUPS - 1:
                flush_act()

    _multiwait_split(nc)
    return nc


_NC_CACHE = None


def _get_nc():
    global _NC_CACHE
    if _NC_CACHE is None:
        _NC_CACHE = build_nc()
    return _NC_CACHE


def _run(inputs, **run_kwargs):
    import ml_dtypes

    nc = _get_nc()
    bf16 = ml_dtypes.bfloat16
    z = np.ascontiguousarray(np.asarray(inputs["z"], dtype=np.float32))
    a0 = np.asarray(inputs["a_0"], dtype=np.float32).reshape(1)
    bk = np.asarray(inputs["bk"], dtype=np.float32).reshape(64)
    ck = np.ascontiguousarray(np.asarray(inputs["ck"], dtype=np.float32))
    dk = np.asarray(inputs["dk"], dtype=np.float32).reshape(64)

    # Host-side constant prep (tiny): block-diagonal ck stationary, wide bk
    # stationary, replicated dk bias column, broadcast a0.
    ckbd = np.zeros((128, 128), dtype=bf16)
    for a in range(4):
        for v in range(2):
            ckbd[32 * a + 16 * v: 32 * a + 16 * v + 16,
                 64 * v: 64 * v + 64] = ck.T.astype(bf16)
    bwide = np.zeros((128, 64), dtype=bf16)
    bwide[0:64, 30] = bk.astype(bf16)
    bwide[64:128, 31] = bk.astype(bf16)
    dk_col = np.concatenate([dk, dk]).reshape(128, 1)
    a0_col = np.broadcast_to(a0.reshape(1, 1), (32, 1)).copy()
    in_maps = []
    for c in range(N_CORES):
        in_maps.append({
            "z": z[c * NC_ROWS:(c + 1) * NC_ROWS],
            "ckbd": ckbd, "bwide": bwide,
            "dk_col": dk_col, "a0_col": a0_col,
        })
    res = run_bass_kernel_spmd(nc, in_maps, core_ids=list(range(N_CORES)),
                               **run_kwargs)
    y = np.concatenate([res.results[c]["y"] for c in range(N_CORES)], axis=0)
    return y, res


def kernel(**inputs) -> np.ndarray:
    y, _ = _run(inputs)
    return y


# revision 48
# speedup vs baseline: 1.4688x; 1.0143x over previous
"""Trainium2 Bass kernel for the MAVE global-epistasis measurement layer.

    y[b] = a_0 + sum_k bk[k] * tanh( (ck @ z[b])[k] + dk[k] )
    z: [2097152, 16] f32, ck: [64, 16], bk, dk: [64], a_0: [1]

Data-parallel over 8 NeuronCores (262144 batch rows per core).

Per-core dataflow (Tile kernel), batch enumerated per super s (4096 rows):
  - Constants (block-diagonal ck stationary, sliding-window bk stationary,
    dk bias column, broadcast a0) are precomputed host-side in numpy and
    loaded with 4 contiguous cast-free DMAs.
  - z loaded+cast f32->bf16 by gpsimd-initiated DMA into [128, 512] tiles,
    partition p = 32 consecutive rows (2 KB contiguous per partition).
  - DVE 32x32 block transpose -> zt[32a + 16v + z, 32Q + j] =
    z[4096 s + 1024 a + 32 j + 2 Q + v, z] (bf16).
  - TensorE: 4 concurrent row-tiled matmuls (tile_position=(32a,0), K=32)
    against the block-diagonal ck stationary -> pre-h [128=(64v+k), 512].
  - ScalarE: tanh(x + dk) over [128, 1536] PSUM tiles (3 banks,
    double-buffered; this stream is the kernel's bottleneck) -> bf16.
  - TensorE: bk stationary [128, 32] (sliding window of one wide const)
    reduces k; 16 groups (slots) accumulate into one y PSUM
    [32=(2*slot+v), 512] bank (double-buffered).
  - DVE 32x32-transposes y, then adds a0 while permuting the free dim so
    partition j holds batch rows {1024 slot + 32 j + (0..31)}; HWDGE
    writes 128-byte contiguous DRAM runs.
"""
import numpy as np

import concourse.bass as bass
import concourse.tile as tile
from concourse import mybir
from concourse.bass_utils import run_bass_kernel_spmd

from contextlib import ExitStack

F32 = mybir.dt.float32
BF16 = mybir.dt.bfloat16

B_FULL = 2097152
N_CORES = 8
NC_ROWS = B_FULL // N_CORES          # 262144
SUPER = 4096                         # rows per transpose tile
N_SUPER = NC_ROWS // SUPER           # 64
N_GROUPS = N_SUPER * 4               # 256  (1024 rows each)
SPAN = 16384                         # rows per y flush (16 groups)
N_SPAN = NC_ROWS // SPAN             # 16
HTILE = 3                            # groups per ACT tanh op (3 PSUM banks)


def _multiwait_split(nc):
    ctr = 0
    for f in nc.m.functions:
        for blk in f.blocks:
            insts = blk.instructions
            i = 0
            while i < len(insts):
                inst = insts[i]
                si = getattr(inst, "sync_info", None)
                if si is not None and si.on_wait and len(si.on_wait) > 1:
                    extra = list(si.on_wait[:-1])
                    del si.on_wait[:-1]
                    for w in extra:
                        ctr += 1
                        nop = mybir.InstNoOp(name=f"I-mws-{ctr}", ins=[], outs=[])
                        nop.engine = inst.engine
                        nop.sync_info = mybir.SyncInfo(on_wait=[w], on_update=[])
                        insts.insert(i, nop)
                        i += 1
                i += 1
    return nc


def build_nc():
    nc = bass.Bass()
    z_ext = nc.declare_dram_parameter("z", [NC_ROWS, 16], F32, isOutput=False)
    # Host-precomputed constant tiles (see _run): contiguous, cast-free loads.
    ckbd_ext = nc.declare_dram_parameter("ckbd", [128, 128], BF16, isOutput=False)
    bwide_ext = nc.declare_dram_parameter("bwide", [128, 64], BF16, isOutput=False)
    dkc_ext = nc.declare_dram_parameter("dk_col", [128, 1], F32, isOutput=False)
    a0c_ext = nc.declare_dram_parameter("a0_col", [32, 1], F32, isOutput=False)
    y_ext = nc.declare_dram_parameter("y", [NC_ROWS, 1], F32, isOutput=True)

    ctx = ExitStack()
    with ctx:
        tc = ctx.enter_context(tile.TileContext(nc))
        consts = ctx.enter_context(tc.tile_pool(name="consts", bufs=1))
        zn_pool = ctx.enter_context(tc.tile_pool(name="zn", bufs=4))
        ztr_pool = ctx.enter_context(tc.tile_pool(name="ztr", bufs=4))
        hsb_pool = ctx.enter_context(tc.tile_pool(name="hsb", bufs=3))
        ysb_pool = ctx.enter_context(tc.tile_pool(name="ysb", bufs=2))
        yt_pool = ctx.enter_context(tc.tile_pool(name="yt", bufs=2))
        hps_pool = ctx.enter_context(tc.tile_pool(name="hps", bufs=2, space="PSUM"))
        yps_pool = ctx.enter_context(tc.tile_pool(name="yps", bufs=2, space="PSUM"))

        # DRAM views ------------------------------------------------------
        # z input: partition p of super s holds rows [4096 s + 32 p, +32),
        # i.e. 2 KB contiguous per partition.
        zd = z_ext[:].rearrange("(s p r) z -> s p (r z)", s=N_SUPER, p=128, r=32)
        # row = 16384 t + 1024 slot + 32 j + w, where w = 2Q + v
        yd = y_ext[:].rearrange(
            "(t slot j w) one -> t j slot (w one)",
            t=N_SPAN, slot=16, j=32, w=32,
        )

        zb_tiles = {}

        def load_super(s):
            zb = zn_pool.tile([128, 512], BF16, tag="zb")
            nc.gpsimd.dma_start(out=zb, in_=zd[s])
            zb_tiles[s] = zb

        # ---- constants: host-precomputed, 4 contiguous cast-free loads.
        ckbd = consts.tile([128, 128], BF16, tag="ckbd")
        nc.sync.dma_start(out=ckbd, in_=ckbd_ext[:])
        bwide = consts.tile([128, 64], BF16, tag="bwide")
        nc.sync.dma_start(out=bwide, in_=bwide_ext[:])
        # slot s uses bwide cols [30-2s, 62-2s): bk sits at cols 30 (v=0
        # rows 0-63) / 31 (v=1 rows 64-127) -> slice cols 2s/2s+1 -> y
        # partitions 2s/2s+1.
        bw = [bwide[:, 30 - 2 * s: 62 - 2 * s] for s in range(16)]
        dk_col = consts.tile([128, 1], F32, tag="dkcol")
        nc.sync.dma_start(out=dk_col, in_=dkc_ext[:])
        a0_col = consts.tile([32, 1], F32, tag="a0col")
        nc.sync.dma_start(out=a0_col, in_=a0c_ext[:])

        # Force the tanh ACT table load during startup (it is otherwise
        # lazily loaded at the first real activation, ~1.3 us mid-pipeline).
        tanh_warm = consts.tile([128, 1], BF16, tag="tanhwarm")
        nc.scalar.activation(tanh_warm, dk_col,
                             mybir.ActivationFunctionType.Tanh,
                             bias=dk_col, scale=1.0)

        # ---- z prefetch
        for s in range(4):
            load_super(s)

        # ---- main loop ---------------------------------------------------
        h_ps = h_sb = y_ps = None
        pend = []

        def flush_act():
            nonlocal pend, y_ps
            if not pend:
                return
            ncols = len(pend) * 512
            nc.scalar.activation(
                h_sb[:, :ncols], h_ps[:, :ncols],
                mybir.ActivationFunctionType.Tanh,
                bias=dk_col, scale=1.0,
            )
            for gg, col in pend:
                slot = gg % 16
                t = gg // 16
                if slot == 0:
                    y_ps = yps_pool.tile([32, 512], F32, tag="y_ps")
                nc.tensor.matmul(
                    y_ps, bw[slot], h_sb[:, col:col + 512],
                    start=(slot == 0), stop=(slot == 15),
                )
                if slot == 15:
                    # y_tr[j, 32 Q + 2 slot + v] = y_ps[2 slot + v, 32 Q + j]
                    y_tr = yt_pool.tile([32, 512], F32, tag="y_tr")
                    nc.vector.transpose(y_tr, y_ps)
                    # y_fin[j, 32 slot + 2 Q + v] = y_tr[j, 32 Q + 2 slot + v] + a0
                    y_fin = ysb_pool.tile([32, 512], F32, tag="y_fin")
                    nc.vector.tensor_scalar_add(
                        y_fin.rearrange("j (slot Q v) -> j slot Q v",
                                        slot=16, Q=16, v=2),
                        y_tr.rearrange("j (Q slot v) -> j slot Q v",
                                       Q=16, slot=16, v=2),
                        a0_col,
                    )
                    nc.sync.dma_start(
                        out=yd[t],
                        in_=y_fin.rearrange("j (slot w) -> j slot w",
                                            slot=16, w=32),
                    )
            pend = []

        for g in range(N_GROUPS):
            s, a = divmod(g, 4)
            if a == 0:
                if s not in zb_tiles:
                    load_super(s)
                zb = zb_tiles.pop(s)
                if s + 4 < N_SUPER:
                    load_super(s + 4)
                zt = ztr_pool.tile([128, 512], BF16)
                nc.vector.transpose(zt, zb)
            if g % HTILE == 0:
                h_ps = hps_pool.tile([128, HTILE * 512], F32)
                h_sb = hsb_pool.tile([128, HTILE * 512], BF16)
            col = (g % HTILE) * 512
            nc.tensor.matmul(
                h_ps[:, col:col + 512],
                ckbd[32 * a:32 * a + 32, :],
                zt[32 * a:32 * a + 32, :],
                start=True, stop=True,
                tile_position=(32 * a, 0),
            )
            pend.append((g, col))
            if g % HTILE == HTILE - 1 or g == N_GROUPS - 1:
                flush_act()

    _multiwait_split(nc)
    return nc


_NC_CACHE = None


def _get_nc():
    global _NC_CACHE
    if _NC_CACHE is None:
        _NC_CACHE = build_nc()
    return _NC_CACHE


def _run(inputs, **run_kwargs):
    import ml_dtypes

    nc = _get_nc()
    bf16 = ml_dtypes.bfloat16
    z = np.ascontiguousarray(np.asarray(inputs["z"], dtype=np.float32))
    a0 = np.asarray(inputs["a_0"], dtype=np.float32).reshape(1)
    bk = np.asarray(inputs["bk"], dtype=np.float32).reshape(64)
    ck = np.ascontiguousarray(np.asarray(inputs["ck"], dtype=np.float32))
    dk = np.asarray(inputs["dk"], dtype=np.float32).reshape(64)

    # Host-side constant prep (tiny): block-diagonal ck stationary, wide bk
    # stationary, replicated dk bias column, broadcast a0.
    ckbd = np.zeros((128, 128), dtype=bf16)
    for a in range(4):
        for v in range(2):
            ckbd[32 * a + 16 * v: 32 * a + 16 * v + 16,
                 64 * v: 64 * v + 64] = ck.T.astype(bf16)
    bwide = np.zeros((128, 64), dtype=bf16)
    bwide[0:64, 30] = bk.astype(bf16)
    bwide[64:128, 31] = bk.astype(bf16)
    dk_col = np.concatenate([dk, dk]).reshape(128, 1)
    a0_col = np.broadcast_to(a0.reshape(1, 1), (32, 1)).copy()
    in_maps = []
    for c in range(N_CORES):
        in_maps.append({
            "z": z[c * NC_ROWS:(c + 1) * NC_ROWS],
            "ckbd": ckbd, "bwide": bwide,
            "dk_col": dk_col, "a0_col": a0_col,
        })
    res = run_bass_kernel_spmd(nc, in_maps, core_ids=list(range(N_CORES)),
                               **run_kwargs)
    y = np.concatenate([res.results[c]["y"] for c in range(N_CORES)], axis=0)
    return y, res


def kernel(**inputs) -> np.ndarray:
    y, _ = _run(inputs)
    return y
